# revision 3
# baseline (speedup 1.0000x reference)
"""Trainium2 Bass kernel for a 2-layer LSTM encoder (relu cell activation). v2

Problem: x[128, 512, 64] -> LSTM(256, relu, seq) -> LSTM(128, relu, last) -> out[128, 128]

Strategy (per core, data-parallel over batch, 16 rows/core), changes vs v1:
  - fp8e4m3 recurrent weights (U1, U2, identity) x bf16 moving: halves
    LDWEIGHTS (the per-step floor). Weights scaled by S=16 so all values sit
    in fp8 normal range; compensated by storing h/S in the h ring (folded
    into the o*c multiply via scalar_tensor_tensor at zero cost) and scaling
    W2 by S on the host.
  - relu(c) deleted: c = f*c + i*g with f,i>0, g>=0, c0=0 => c >= 0 always,
    so h = o*relu(c) = o*c.
  - PSUM banks split IF / O / G per step (+ shared bulk pool = 8 banks):
    sigmoid(i,f) starts when the IF bank is done instead of after all MMs.
  - G-gate x-inject eliminated: g_full = psum_g + zx_g computed by one DVE
    scalar_tensor_tensor; relu(g) folded into the i*g multiply (max-then-mult;
    max is a no-op on the c half since c >= 0).
  - x pre-transposed on host to [F, T*B] bf16, resident in SBUF: no PE
    transposes / copies in the bulk path.
  - bulk matmuls + PSUM->SBUF copies emitted interleaved inside the step loop
    (and l2x split in time-halves) so they fill PE/ACT/DVE gaps instead of
    stalling chunk starts.
  - h write split kc0 / (kc1+L2) so next step's first matmuls start earlier.
"""

import numpy as np
from contextlib import ExitStack

import concourse.bass as bass
import concourse.tile as tile
from concourse import bacc
from concourse import mybir
from concourse.bass_utils import run_bass_kernel_spmd

fp32 = mybir.dt.float32
bf16 = mybir.dt.bfloat16
fp8 = mybir.dt.float8e4
AF = mybir.ActivationFunctionType
AOP = mybir.AluOpType

B, T, F = 128, 512, 64
U1, U2 = 256, 128
NCORES = 8
BL = B // NCORES  # 16 batch rows per core
S = 16.0  # fp8 weight scale

# gate block order in fused layout: I, F, O, G. Keras weight column order is
# i, f, g, o -> column offsets per block:
COLMAP1 = [0 * U1, 1 * U1, 3 * U1, 2 * U1]
COLMAP2 = [0 * U2, 1 * U2, 3 * U2, 2 * U2]

CB8_COLS = 8 * U1 + 4 * U2 + 128   # u1q(2x1024) | u2q(512) | id8(128)
CB16_COLS = 8 * U2 + 1024          # w2q(2x512) | w1(1024, rows 0:64)
CF32_COLS = 128 + 8 + 4            # idf(128) | b1p(8) | b2p(4)


def build(T_=T, CH=32, nonzero_bias=False, reps=1):
    assert T_ % CH == 0 and CH % 2 == 0
    NCH = T_ // CH
    HCH = CH // 2
    RING = 3 * CH
    LAG = CH
    TOT = T_ + LAG

    nc = bacc.Bacc("TRN2", target_bir_lowering=False, debug=False)

    xt_d = nc.declare_dram_parameter("xt", [F, T_ * BL], bf16, isOutput=False)
    c8_d = nc.declare_dram_parameter("cb8", [128, CB8_COLS], fp8, isOutput=False)
    cb_d = nc.declare_dram_parameter("cb16", [128, CB16_COLS], bf16, isOutput=False)
    cf_d = nc.declare_dram_parameter("cf32", [128, CF32_COLS], fp32, isOutput=False)
    out_d = nc.declare_dram_parameter("out", [BL, U2], fp32, isOutput=True)

    with tile.TileContext(nc) as tc, ExitStack() as ctx:
        const_p = ctx.enter_context(tc.tile_pool(name="const", bufs=1))
        zx_p = ctx.enter_context(tc.tile_pool(name="zx", bufs=3))
        ew_p = ctx.enter_context(tc.tile_pool(name="ew", bufs=3))
        state_p = ctx.enter_context(tc.tile_pool(name="state", bufs=1))
        pif_p = ctx.enter_context(tc.tile_pool(name="pif", bufs=2, space="PSUM"))
        po_p = ctx.enter_context(tc.tile_pool(name="po", bufs=2, space="PSUM"))
        pg_p = ctx.enter_context(tc.tile_pool(name="pg", bufs=2, space="PSUM"))
        pb_p = ctx.enter_context(tc.tile_pool(name="pb", bufs=2, space="PSUM"))

        # ---- constants (one DMA each) ----
        c8 = const_p.tile([128, CB8_COLS], fp8, name="c8")
        nc.sync.dma_start(c8[:, :], c8_d[:, :])
        cb = const_p.tile([128, CB16_COLS], bf16, name="cb")
        nc.sync.dma_start(cb[:, :], cb_d[:, :])
        cf = const_p.tile([128, CF32_COLS], fp32, name="cf")
        nc.sync.dma_start(cf[:, :], cf_d[:, :])
        xt = const_p.tile([F, T_ * BL], bf16, name="xt")
        nc.sync.dma_start(xt[:, :], xt_d[:, :])

        u1q = [c8[:, 0:4 * U1], c8[:, 4 * U1:8 * U1]]
        u2q = c8[0:U2, 8 * U1:8 * U1 + 4 * U2]
        id8 = c8[:, 8 * U1 + 4 * U2:8 * U1 + 4 * U2 + 128]
        w2q = [cb[:, 0:4 * U2], cb[:, 4 * U2:8 * U2]]
        w1sb = cb[0:F, 8 * U2:8 * U2 + 1024]
        idf = cf[:, 0:128]
        b1sb = cf[:, 128:136]
        b2sb = cf[:, 136:140]

        # ---- persistent state ----
        # gc: [128, 2, 48] = [g_full | c]; c >= 0 invariant lets one stt
        # (max 0, mult) compute [relu(g)*i | c*f]
        gc = state_p.tile([128, 2, 48], fp32)
        h_ring = state_p.tile([128, RING, 48], bf16)  # stores h/S (bf16)

        tc.strict_bb_all_engine_barrier()

        zx_tiles = [None] * (NCH + 1)

        def _get_zx(k):
            if zx_tiles[k] is None:
                z = zx_p.tile([128, CH, 192], bf16, name="zx", tag="zx")
                zx_tiles[k] = z
                if k == 0 or k >= NCH:
                    nc.vector.memset(z[:, :, :], 0.0)
            return zx_tiles[k]

        def _copy(j, dst, src, bias_ap):
            if nonzero_bias:
                nc.vector.tensor_scalar_add(dst, src, bias_ap)
            elif j % 2 == 0:
                nc.scalar.copy(dst, src)
            else:
                nc.vector.tensor_copy(dst, src)

        def l1x_piece(k, p):
            """x-part of L1 gates for chunk k, piece p = (bi, uc)."""
            bi, uc = p // 2, p % 2
            zk = _get_zx(k)
            cc = COLMAP1[bi] + uc * 128
            pb = pb_p.tile([128, CH * BL], fp32, name="pb", tag="pb")
            nc.tensor.matmul(
                pb[:, :], w1sb[:, cc:cc + 128],
                xt[:, k * CH * BL:(k + 1) * CH * BL],
                start=True, stop=True)
            _copy(p, zk[:, :, bi * 48 + uc * 16:bi * 48 + (uc + 1) * 16],
                  pb.rearrange("p (t b) -> p t b", b=BL),
                  b1sb[:, bi * 2 + uc:bi * 2 + uc + 1])

        def l2x_half(j, sj):
            """W2.T @ h1[chunk j, half sj] -> zx[j+1] L2 cols, steps half sj."""
            zk = _get_zx(j + 1)
            rs = (j * CH + sj * HCH) % RING
            for bi in range(4):
                pb = pb_p.tile([128, HCH * BL], fp32, name="pb2", tag="pb")
                for kc in range(2):
                    nc.tensor.matmul(
                        pb[:, :],
                        w2q[kc][:, COLMAP2[bi]:COLMAP2[bi] + 128],
                        h_ring[:, rs:rs + HCH, kc * 16:(kc + 1) * 16],
                        start=(kc == 0), stop=(kc == 1))
                _copy(bi, zk[:, sj * HCH:(sj + 1) * HCH,
                             bi * 48 + 32:bi * 48 + 48],
                      pb.rearrange("p (t b) -> p t b", b=BL),
                      b2sb[:, bi:bi + 1])

        def emit_body():
            nonlocal h2f
            zx_tiles[:] = [None] * (NCH + 1)
            for p in range(8):
                l1x_piece(0, p)
            for t in range(TOT):
                k, tl = divmod(t, CH)
                s = t - LAG  # layer-2 step
                # interleaved bulk work (tl==1 not 0: the l2x moving slice
                # includes h(t-1), so at tl==0 it would delay the step MMs)
                if tl == 1 and 1 <= k <= NCH:
                    l2x_half(k - 1, 1)
                if tl == HCH + 4 and k < NCH:
                    l2x_half(k, 0)
                if tl in (2, 4, 6, 8, 10, 12, 14, 16) and k + 1 < NCH:
                    l1x_piece(k + 1, (tl - 2) // 2)

                zxt = zx_tiles[k]
                hp = h_ring[:, (t - 1) % RING, :]
                pif = pif_p.tile([128, 2, 48], fp32, name="pif")
                po = po_p.tile([128, 48], fp32, name="po")
                pg = pg_p.tile([128, 48], fp32, name="pg")

                # accumulation groups; injects first (no h dependency -> they
                # prefire during the previous step's elementwise tail)
                ifs = [(pif[:, :, :], id8[:, :], zxt[:, tl, 0:96])]
                os_ = [(po[:, :], id8[:, :], zxt[:, tl, 96:144])]
                gs = []
                if t < T_:
                    for kc in range(2):
                        for bi, gl in ((0, ifs), (1, ifs), (2, os_), (3, gs)):
                            for uc in range(2):
                                cc = COLMAP1[bi] + uc * 128
                                o_ap = (pif[:, bi, uc * 16:(uc + 1) * 16] if bi < 2
                                        else (po if bi == 2 else pg)[:, uc * 16:(uc + 1) * 16])
                                gl.append((o_ap, u1q[kc][:, cc:cc + 128],
                                           hp[:, kc * 16:(kc + 1) * 16]))
                if s >= 0:
                    for bi, gl in ((0, ifs), (1, ifs), (2, os_), (3, gs)):
                        o_ap = (pif[:, bi, 32:48] if bi < 2
                                else (po if bi == 2 else pg)[:, 32:48])
                        gl.append((o_ap, u2q[:, COLMAP2[bi]:COLMAP2[bi] + 128],
                                   hp[:, 32:48]))
                # emission order: both injects first (no h dep -> prefire
                # during previous tail), then IF MMs, G MMs, O MMs. start/stop
                # flags are per accumulation group (per psum tile).
                seq = [(ifs[0], True, len(ifs) == 1),
                       (os_[0], True, len(os_) == 1)]
                seq += [(m, False, i == len(ifs) - 2)
                        for i, m in enumerate(ifs[1:])]
                seq += [(m, i == 0, i == len(gs) - 1)
                        for i, m in enumerate(gs)]
                seq += [(m, False, i == len(os_) - 2)
                        for i, m in enumerate(os_[1:])]
                for (o, l, r), st, sp in seq:
                    nc.tensor.matmul(o, l, r, start=st, stop=sp)

                # elementwise tail
                gif = ew_p.tile([128, 2, 48], fp32, name="gif")
                go = ew_p.tile([128, 48], fp32, name="go")
                # g_full = psum_g + zx_g (no inject matmul for G)
                nc.vector.scalar_tensor_tensor(
                    gc[:, 0, :], pg[:, :], 0.0, zxt[:, tl, 144:192],
                    AOP.add, AOP.add)
                nc.scalar.activation(gif[:, :, :], pif[:, :, :], AF.Sigmoid)
                nc.scalar.activation(go[:, :], po[:, :], AF.Sigmoid)
                # [i*relu(g) | f*c] in one op (max 0 is a no-op on c)
                igfc = ew_p.tile([128, 2, 48], fp32, name="igfc")
                nc.vector.scalar_tensor_tensor(
                    igfc[:, :, :], gc[:, :, :], 0.0, gif[:, :, :],
                    AOP.max, AOP.mult)
                nc.vector.tensor_add(gc[:, 1, :], igfc[:, 0, :], igfc[:, 1, :])
                slot = t % RING
                # h/S = (o * 1/S) * c, kc0 first so next step's MMs can start
                nc.vector.scalar_tensor_tensor(
                    h_ring[:, slot, 0:16], go[:, 0:16], 1.0 / S,
                    gc[:, 1, 0:16], AOP.mult, AOP.mult)
                nc.vector.scalar_tensor_tensor(
                    h_ring[:, slot, 16:48], go[:, 16:48], 1.0 / S,
                    gc[:, 1, 16:48], AOP.mult, AOP.mult)

                if t == LAG - 1:
                    # reset L2 state before its first real step
                    nc.vector.memset(h_ring[:, slot, 32:48], 0.0)
                    nc.vector.memset(gc[:, 1, 32:48], 0.0)
                if t == TOT - 1:
                    h2f = ew_p.tile([128, BL], fp32, name="h2f")
                    nc.vector.tensor_mul(h2f[:, :], go[:, 32:48], gc[:, 1, 32:48])

        h2f = None
        for _rep in range(reps):
            nc.vector.memset(gc[:, :, :], 0.0)
            nc.vector.memset(h_ring[:, RING - 1, :], 0.0)
            emit_body()

        pfin = pb_p.tile([BL, 128], fp32, name="pfin", tag="pb")
        nc.tensor.transpose(pfin[:, :], h2f[:, :], idf[:, :])
        osb = ew_p.tile([BL, 128], fp32, name="osb")
        nc.scalar.copy(osb[:, :], pfin[:, :])
        nc.sync.dma_start(out_d[:, :], osb[:, :])

    nc.finalize()
    return nc


_cache = {}


def _get_nc(T_=T, CH=32, nonzero_bias=False, reps=1):
    key = (T_, CH, nonzero_bias, reps)
    if key not in _cache:
        _cache[key] = build(T_, CH, nonzero_bias, reps)
    return _cache[key]


def make_inputs(x, W1, U1w, b1, W2, U2w, b2, T_=T):
    np8 = mybir.dt.np(fp8)
    npb = mybir.dt.np(bf16)
    x = np.asarray(x, np.float32)
    W1 = np.asarray(W1, np.float32)
    U1w = np.asarray(U1w, np.float32)
    W2 = np.asarray(W2, np.float32)
    U2w = np.asarray(U2w, np.float32)
    b1 = np.asarray(b1, np.float32)
    b2 = np.asarray(b2, np.float32)

    cb8 = np.zeros((128, CB8_COLS), np8)
    u1q = (U1w * S).astype(np8)
    cb8[:, 0:1024] = u1q[0:128]
    cb8[:, 1024:2048] = u1q[128:256]
    cb8[:, 2048:2560] = (U2w * S).astype(np8)
    cb8[:, 2560:2688] = np.eye(128, dtype=np.float32).astype(np8)

    cb16 = np.zeros((128, CB16_COLS), npb)
    w2q = (W2 * S).astype(npb)
    cb16[:, 0:512] = w2q[0:128]
    cb16[:, 512:1024] = w2q[128:256]
    cb16[0:64, 1024:2048] = W1.astype(npb)

    b1p = np.zeros((128, 8), np.float32)
    for bi in range(4):
        for uc in range(2):
            b1p[:, bi * 2 + uc] = b1[COLMAP1[bi] + uc * 128:COLMAP1[bi] + (uc + 1) * 128]
    b2p = np.zeros((128, 4), np.float32)
    for bi in range(4):
        b2p[:, bi] = b2[COLMAP2[bi]:COLMAP2[bi] + 128]
    cf32 = np.zeros((128, CF32_COLS), np.float32)
    cf32[:, 0:128] = np.eye(128, dtype=np.float32)
    cf32[:, 128:136] = b1p
    cf32[:, 136:140] = b2p

    common = dict(cb8=cb8, cb16=cb16, cf32=cf32)
    xr = x.reshape(NCORES, BL, x.shape[1], F)
    in_maps = []
    for c in range(NCORES):
        xtc = np.ascontiguousarray(
            xr[c][:, :T_].transpose(2, 1, 0).reshape(F, T_ * BL)).astype(npb)
        m = dict(common)
        m["xt"] = xtc
        in_maps.append(m)
    nonzero_bias = bool(np.any(b1) or np.any(b2))
    return in_maps, nonzero_bias


def run(inputs, T_=T, CH=32, trace=False, reps=1):
    in_maps, nzb = make_inputs(
        inputs["x"], inputs["W1"], inputs["U1"], inputs["b1"],
        inputs["W2"], inputs["U2"], inputs["b2"], T_=T_)
    nc = _get_nc(T_, CH, nzb, reps)
    res = run_bass_kernel_spmd(nc, in_maps, list(range(NCORES)), trace=trace)
    out = np.concatenate(
        [res.results[c]["out"] for c in range(NCORES)], axis=0)
    return np.ascontiguousarray(out, dtype=np.float32), res.exec_time_ns


def kernel(x, W1, U1, b1, W2, U2, b2):
    out, _ = run(dict(x=x, W1=W1, U1=U1, b1=b1, W2=W2, U2=U2, b2=b2))
    return out


# revision 4
# speedup vs baseline: 3.7202x; 3.7202x over previous
"""Trainium2 Bass kernel for a 2-layer LSTM encoder (relu cell activation). v2

Problem: x[128, 512, 64] -> LSTM(256, relu, seq) -> LSTM(128, relu, last) -> out[128, 128]

Strategy (per core, data-parallel over batch, 16 rows/core), changes vs v1:
  - fp8e4m3 recurrent weights (U1, U2, identity) x bf16 moving: halves
    LDWEIGHTS (the per-step floor). Weights scaled by S=16 so all values sit
    in fp8 normal range; compensated by storing h/S in the h ring (folded
    into the o*c multiply via scalar_tensor_tensor at zero cost) and scaling
    W2 by S on the host.
  - relu(c) deleted: c = f*c + i*g with f,i>0, g>=0, c0=0 => c >= 0 always,
    so h = o*relu(c) = o*c.
  - PSUM banks split IF / O / G per step (+ shared bulk pool = 8 banks):
    sigmoid(i,f) starts when the IF bank is done instead of after all MMs.
  - G-gate x-inject eliminated: g_full = psum_g + zx_g computed by one DVE
    scalar_tensor_tensor; relu(g) folded into the i*g multiply (max-then-mult;
    max is a no-op on the c half since c >= 0).
  - x pre-transposed on host to [F, T*B] bf16, resident in SBUF: no PE
    transposes / copies in the bulk path.
  - bulk matmuls + PSUM->SBUF copies emitted interleaved inside the step loop
    (and l2x split in time-halves) so they fill PE/ACT/DVE gaps instead of
    stalling chunk starts.
  - h write split kc0 / (kc1+L2) so next step's first matmuls start earlier.
"""

import numpy as np
from contextlib import ExitStack

import concourse.bass as bass
import concourse.tile as tile
from concourse import bacc
from concourse import mybir
from concourse.bass_utils import run_bass_kernel_spmd

fp32 = mybir.dt.float32
bf16 = mybir.dt.bfloat16
fp8 = mybir.dt.float8e4
AF = mybir.ActivationFunctionType
AOP = mybir.AluOpType

B, T, F = 128, 512, 64
U1, U2 = 256, 128
NCORES = 8
BL = B // NCORES  # 16 batch rows per core
S = 16.0  # fp8 weight scale

# gate block order in fused layout: I, F, O, G. Keras weight column order is
# i, f, g, o -> column offsets per block:
COLMAP1 = [0 * U1, 1 * U1, 3 * U1, 2 * U1]
COLMAP2 = [0 * U2, 1 * U2, 3 * U2, 2 * U2]

CB8_COLS = 8 * U1 + 4 * U2 + 128   # u1q(2x1024) | u2q(512) | id8(128)
CB16_COLS = 8 * U2 + 1024          # w2q(2x512) | w1(1024, rows 0:64)
CF32_COLS = 128 + 8 + 4            # idf(128) | b1p(8) | b2p(4)


def build(T_=T, CH=32, nonzero_bias=False, reps=1):
    assert T_ % CH == 0 and CH % 2 == 0
    NCH = T_ // CH
    HCH = CH // 2
    RING = 3 * CH
    LAG = CH
    TOT = T_ + LAG

    nc = bacc.Bacc("TRN2", target_bir_lowering=False, debug=False)

    xt_d = nc.declare_dram_parameter("xt", [F, T_ * BL], bf16, isOutput=False)
    c8_d = nc.declare_dram_parameter("cb8", [128, CB8_COLS], fp8, isOutput=False)
    cb_d = nc.declare_dram_parameter("cb16", [128, CB16_COLS], bf16, isOutput=False)
    cf_d = nc.declare_dram_parameter("cf32", [128, CF32_COLS], fp32, isOutput=False)
    out_d = nc.declare_dram_parameter("out", [BL, U2], fp32, isOutput=True)

    with tile.TileContext(nc) as tc, ExitStack() as ctx:
        const_p = ctx.enter_context(tc.tile_pool(name="const", bufs=1))
        zx_p = ctx.enter_context(tc.tile_pool(name="zx", bufs=3))
        ew_p = ctx.enter_context(tc.tile_pool(name="ew", bufs=3))
        state_p = ctx.enter_context(tc.tile_pool(name="state", bufs=1))
        pif_p = ctx.enter_context(tc.tile_pool(name="pif", bufs=2, space="PSUM"))
        po_p = ctx.enter_context(tc.tile_pool(name="po", bufs=2, space="PSUM"))
        pg_p = ctx.enter_context(tc.tile_pool(name="pg", bufs=2, space="PSUM"))
        pb_p = ctx.enter_context(tc.tile_pool(name="pb", bufs=2, space="PSUM"))

        # ---- constants (one DMA each) ----
        c8 = const_p.tile([128, CB8_COLS], fp8, name="c8")
        nc.sync.dma_start(c8[:, :], c8_d[:, :])
        cb = const_p.tile([128, CB16_COLS], bf16, name="cb")
        nc.sync.dma_start(cb[:, :], cb_d[:, :])
        cf = const_p.tile([128, CF32_COLS], fp32, name="cf")
        nc.sync.dma_start(cf[:, :], cf_d[:, :])
        xt = const_p.tile([F, T_ * BL], bf16, name="xt")
        nc.sync.dma_start(xt[:, :], xt_d[:, :])

        u1q = [c8[:, 0:4 * U1], c8[:, 4 * U1:8 * U1]]
        u2q = c8[0:U2, 8 * U1:8 * U1 + 4 * U2]
        id8 = c8[:, 8 * U1 + 4 * U2:8 * U1 + 4 * U2 + 128]
        w2q = [cb[:, 0:4 * U2], cb[:, 4 * U2:8 * U2]]
        w1sb = cb[0:F, 8 * U2:8 * U2 + 1024]
        idf = cf[:, 0:128]
        b1sb = cf[:, 128:136]
        b2sb = cf[:, 136:140]

        # ---- persistent state ----
        # gc: [128, 2, 48] = [g_full | c]; c >= 0 invariant lets one stt
        # (max 0, mult) compute [relu(g)*i | c*f]
        gc = state_p.tile([128, 2, 48], fp32)
        h_ring = state_p.tile([128, RING, 48], bf16)  # stores h/S (bf16)

        tc.strict_bb_all_engine_barrier()

        zx_tiles = [None] * (NCH + 1)

        def _get_zx(k):
            if zx_tiles[k] is None:
                z = zx_p.tile([128, CH, 192], bf16, name="zx", tag="zx")
                zx_tiles[k] = z
                if k == 0 or k >= NCH:
                    nc.vector.memset(z[:, :, :], 0.0)
            return zx_tiles[k]

        def _copy(j, dst, src, bias_ap):
            if nonzero_bias:
                nc.vector.tensor_scalar_add(dst, src, bias_ap)
            elif j % 2 == 0:
                nc.scalar.copy(dst, src)
            else:
                nc.vector.tensor_copy(dst, src)

        def l1x_piece(k, p):
            """x-part of L1 gates for chunk k, piece p = (bi, uc). Split into
            half-chunk sub-MMs/copies so each engine-FIFO item stays small and
            cannot delay the recurrence chain ops behind it for long."""
            bi, uc = p // 2, p % 2
            zk = _get_zx(k)
            cc = COLMAP1[bi] + uc * 128
            for sj in range(2):
                pb = pb_p.tile([128, HCH * BL], fp32, name="pb", tag="pb")
                nc.tensor.matmul(
                    pb[:, :], w1sb[:, cc:cc + 128],
                    xt[:, (k * CH + sj * HCH) * BL:(k * CH + (sj + 1) * HCH) * BL],
                    start=True, stop=True)
                _copy(p + sj, zk[:, sj * HCH:(sj + 1) * HCH,
                                 bi * 48 + uc * 16:bi * 48 + (uc + 1) * 16],
                      pb.rearrange("p (t b) -> p t b", b=BL),
                      b1sb[:, bi * 2 + uc:bi * 2 + uc + 1])

        def l2x_half(j, sj):
            """W2.T @ h1[chunk j, half sj] -> zx[j+1] L2 cols, steps half sj."""
            zk = _get_zx(j + 1)
            rs = (j * CH + sj * HCH) % RING
            for bi in range(4):
                pb = pb_p.tile([128, HCH * BL], fp32, name="pb2", tag="pb")
                for kc in range(2):
                    nc.tensor.matmul(
                        pb[:, :],
                        w2q[kc][:, COLMAP2[bi]:COLMAP2[bi] + 128],
                        h_ring[:, rs:rs + HCH, kc * 16:(kc + 1) * 16],
                        start=(kc == 0), stop=(kc == 1))
                _copy(bi, zk[:, sj * HCH:(sj + 1) * HCH,
                             bi * 48 + 32:bi * 48 + 48],
                      pb.rearrange("p (t b) -> p t b", b=BL),
                      b2sb[:, bi:bi + 1])

        def emit_body():
            nonlocal h2f
            zx_tiles[:] = [None] * (NCH + 1)
            for p in range(8):
                l1x_piece(0, p)
            for t in range(TOT):
                k, tl = divmod(t, CH)
                s = t - LAG  # layer-2 step
                # interleaved bulk work (tl==1 not 0: the l2x moving slice
                # includes h(t-1), so at tl==0 it would delay the step MMs)
                if tl == 1 and 1 <= k <= NCH:
                    l2x_half(k - 1, 1)
                if tl == HCH + 4 and k < NCH:
                    l2x_half(k, 0)
                if tl in (2, 4, 6, 8, 10, 12, 14, 16) and k + 1 < NCH:
                    l1x_piece(k + 1, (tl - 2) // 2)

                zxt = zx_tiles[k]
                hp = h_ring[:, (t - 1) % RING, :]
                pif = pif_p.tile([128, 2, 48], fp32, name="pif")
                po = po_p.tile([128, 48], fp32, name="po")
                pg = pg_p.tile([128, 48], fp32, name="pg")

                # accumulation groups; injects first (no h dependency -> they
                # prefire during the previous step's elementwise tail)
                ifs = [(pif[:, :, :], id8[:, :], zxt[:, tl, 0:96])]
                os_ = [(po[:, :], id8[:, :], zxt[:, tl, 96:144])]
                gs = []
                if t < T_:
                    for kc in range(2):
                        for bi, gl in ((0, ifs), (1, ifs), (2, os_), (3, gs)):
                            for uc in range(2):
                                cc = COLMAP1[bi] + uc * 128
                                o_ap = (pif[:, bi, uc * 16:(uc + 1) * 16] if bi < 2
                                        else (po if bi == 2 else pg)[:, uc * 16:(uc + 1) * 16])
                                gl.append((o_ap, u1q[kc][:, cc:cc + 128],
                                           hp[:, kc * 16:(kc + 1) * 16]))
                if s >= 0:
                    for bi, gl in ((0, ifs), (1, ifs), (2, os_), (3, gs)):
                        o_ap = (pif[:, bi, 32:48] if bi < 2
                                else (po if bi == 2 else pg)[:, 32:48])
                        gl.append((o_ap, u2q[:, COLMAP2[bi]:COLMAP2[bi] + 128],
                                   hp[:, 32:48]))
                # emission order: both injects first (no h dep -> prefire
                # during previous tail), then IF MMs, G MMs, O MMs. start/stop
                # flags are per accumulation group (per psum tile).
                seq = [(ifs[0], True, len(ifs) == 1),
                       (os_[0], True, len(os_) == 1)]
                seq += [(m, False, i == len(ifs) - 2)
                        for i, m in enumerate(ifs[1:])]
                seq += [(m, i == 0, i == len(gs) - 1)
                        for i, m in enumerate(gs)]
                seq += [(m, False, i == len(os_) - 2)
                        for i, m in enumerate(os_[1:])]
                for (o, l, r), st, sp in seq:
                    nc.tensor.matmul(o, l, r, start=st, stop=sp)

                # elementwise tail
                gif = ew_p.tile([128, 2, 48], fp32, name="gif")
                go = ew_p.tile([128, 48], fp32, name="go")
                # g_full = psum_g + zx_g (no inject matmul for G)
                nc.vector.scalar_tensor_tensor(
                    gc[:, 0, :], pg[:, :], 0.0, zxt[:, tl, 144:192],
                    AOP.add, AOP.add)
                nc.scalar.activation(gif[:, :, :], pif[:, :, :], AF.Sigmoid)
                nc.scalar.activation(go[:, :], po[:, :], AF.Sigmoid)
                # [i*relu(g) | f*c] in one op (max 0 is a no-op on c)
                igfc = ew_p.tile([128, 2, 48], fp32, name="igfc")
                nc.vector.scalar_tensor_tensor(
                    igfc[:, :, :], gc[:, :, :], 0.0, gif[:, :, :],
                    AOP.max, AOP.mult)
                nc.vector.tensor_add(gc[:, 1, :], igfc[:, 0, :], igfc[:, 1, :])
                slot = t % RING
                # h/S = (o * 1/S) * c, kc0 first so next step's MMs can start
                nc.vector.scalar_tensor_tensor(
                    h_ring[:, slot, 0:16], go[:, 0:16], 1.0 / S,
                    gc[:, 1, 0:16], AOP.mult, AOP.mult)
                nc.vector.scalar_tensor_tensor(
                    h_ring[:, slot, 16:48], go[:, 16:48], 1.0 / S,
                    gc[:, 1, 16:48], AOP.mult, AOP.mult)

                if t == LAG - 1:
                    # reset L2 state before its first real step
                    nc.vector.memset(h_ring[:, slot, 32:48], 0.0)
                    nc.vector.memset(gc[:, 1, 32:48], 0.0)
                if t == TOT - 1:
                    h2f = ew_p.tile([128, BL], fp32, name="h2f")
                    nc.vector.tensor_mul(h2f[:, :], go[:, 32:48], gc[:, 1, 32:48])

        h2f = None
        for _rep in range(reps):
            nc.vector.memset(gc[:, :, :], 0.0)
            nc.vector.memset(h_ring[:, RING - 1, :], 0.0)
            emit_body()

        pfin = pb_p.tile([BL, 128], fp32, name="pfin", tag="pb")
        nc.tensor.transpose(pfin[:, :], h2f[:, :], idf[:, :])
        osb = ew_p.tile([BL, 128], fp32, name="osb")
        nc.scalar.copy(osb[:, :], pfin[:, :])
        nc.sync.dma_start(out_d[:, :], osb[:, :])

    nc.finalize()
    return nc


_cache = {}


def _get_nc(T_=T, CH=32, nonzero_bias=False, reps=1):
    key = (T_, CH, nonzero_bias, reps)
    if key not in _cache:
        _cache[key] = build(T_, CH, nonzero_bias, reps)
    return _cache[key]


def make_inputs(x, W1, U1w, b1, W2, U2w, b2, T_=T):
    np8 = mybir.dt.np(fp8)
    npb = mybir.dt.np(bf16)
    x = np.asarray(x, np.float32)
    W1 = np.asarray(W1, np.float32)
    U1w = np.asarray(U1w, np.float32)
    W2 = np.asarray(W2, np.float32)
    U2w = np.asarray(U2w, np.float32)
    b1 = np.asarray(b1, np.float32)
    b2 = np.asarray(b2, np.float32)

    cb8 = np.zeros((128, CB8_COLS), np8)
    u1q = (U1w * S).astype(np8)
    cb8[:, 0:1024] = u1q[0:128]
    cb8[:, 1024:2048] = u1q[128:256]
    cb8[:, 2048:2560] = (U2w * S).astype(np8)
    cb8[:, 2560:2688] = np.eye(128, dtype=np.float32).astype(np8)

    cb16 = np.zeros((128, CB16_COLS), npb)
    w2q = (W2 * S).astype(npb)
    cb16[:, 0:512] = w2q[0:128]
    cb16[:, 512:1024] = w2q[128:256]
    cb16[0:64, 1024:2048] = W1.astype(npb)

    b1p = np.zeros((128, 8), np.float32)
    for bi in range(4):
        for uc in range(2):
            b1p[:, bi * 2 + uc] = b1[COLMAP1[bi] + uc * 128:COLMAP1[bi] + (uc + 1) * 128]
    b2p = np.zeros((128, 4), np.float32)
    for bi in range(4):
        b2p[:, bi] = b2[COLMAP2[bi]:COLMAP2[bi] + 128]
    cf32 = np.zeros((128, CF32_COLS), np.float32)
    cf32[:, 0:128] = np.eye(128, dtype=np.float32)
    cf32[:, 128:136] = b1p
    cf32[:, 136:140] = b2p

    common = dict(cb8=cb8, cb16=cb16, cf32=cf32)
    xr = x.reshape(NCORES, BL, x.shape[1], F)
    in_maps = []
    for c in range(NCORES):
        xtc = np.ascontiguousarray(
            xr[c][:, :T_].transpose(2, 1, 0).reshape(F, T_ * BL)).astype(npb)
        m = dict(common)
        m["xt"] = xtc
        in_maps.append(m)
    nonzero_bias = bool(np.any(b1) or np.any(b2))
    return in_maps, nonzero_bias


def run(inputs, T_=T, CH=32, trace=False, reps=1):
    in_maps, nzb = make_inputs(
        inputs["x"], inputs["W1"], inputs["U1"], inputs["b1"],
        inputs["W2"], inputs["U2"], inputs["b2"], T_=T_)
    nc = _get_nc(T_, CH, nzb, reps)
    res = run_bass_kernel_spmd(nc, in_maps, list(range(NCORES)), trace=trace)
    out = np.concatenate(
        [res.results[c]["out"] for c in range(NCORES)], axis=0)
    return np.ascontiguousarray(out, dtype=np.float32), res.exec_time_ns


def kernel(x, W1, U1, b1, W2, U2, b2):
    out, _ = run(dict(x=x, W1=W1, U1=U1, b1=b1, W2=W2, U2=U2, b2=b2))
    return out
